# revision 16
# baseline (speedup 1.0000x reference)
"""Trainium2 Bass kernel for a Mamba-style selective-scan (SSM) layer.

Full shapes: x (4, 2048, 768); d_inner 1536, d_state 16, d_conv 4, dt_rank 48.

Sharding over 8 NeuronCores: core c handles batch b = c//2 and d_inner half
j = c%2 (768 channels).  The x-branch (in_proj x-part, depthwise conv,
x_proj) is duplicated across the two cores of a batch (it is needed in full
for B/C/delta); channels are permuted per core so that the local 768 channels
always occupy tiles 0..5 (keeps the program SPMD-uniform).  Each core's
out_proj partial (768, 2048) is summed host-side across the core pair.

On-chip layout is transposed: d on partitions, L on the free dim.  L is
processed in passes of LP columns; the selective scan runs as hardware
tensor_tensor_scan instructions (state = dA*state + dBu along the free dim),
one per (state index n, d-tile), with the [128, 96] running state chained
across passes via its `initial` operand.
"""

import numpy as np

import concourse.bass as bass
from concourse import bacc
import concourse.mybir as mybir
import concourse.tile as tile
from concourse.bass_utils import run_bass_kernel_spmd

OP = mybir.AluOpType
AF = mybir.ActivationFunctionType
F32 = mybir.dt.float32
F32R = mybir.dt.float32r
BF16 = mybir.dt.bfloat16


class Dims:
    def __init__(self, B=4, L=2048, DM=768, DI=1536, DS=16, DC=4, DR=48, LP=512):
        self.B, self.L, self.DM, self.DI = B, L, DM, DI
        self.DS, self.DC, self.DR, self.LP = DS, DC, DR, LP
        self.DH = DI // 2                  # local d_inner half per core
        self.NP = L // LP                  # number of L passes
        self.KT = DM // 128                # k-tiles of d_model
        self.FT = DI // 128                # full d_inner tiles
        self.DT = self.DH // 128           # local d_inner tiles
        self.MT = (DI + self.DH) // 128    # in_proj output tiles (x part + res)
        self.NT = DS                       # states
        assert DM % 128 == 0 and DI % 128 == 0 and L % LP == 0


def build_nc(d: Dims):
    nc = bacc.Bacc()
    LP = d.LP

    # ---- DRAM parameters (per-core inputs, host-prepacked) ----
    xT = nc.declare_dram_parameter("xT", [d.DM, d.L], F32R, isOutput=False)
    w1 = nc.declare_dram_parameter("w1", [d.DM, d.DI + d.DH], F32R, isOutput=False)
    wx = nc.declare_dram_parameter("wx", [128, d.FT * 80], F32R, isOutput=False)
    wdt = nc.declare_dram_parameter("wdt", [d.DR, d.DH], F32R, isOutput=False)
    wout = nc.declare_dram_parameter("wout", [d.DH, d.DM], F32R, isOutput=False)
    convw = nc.declare_dram_parameter("convw", [128, d.FT * d.DC], F32, isOutput=False)
    convb = nc.declare_dram_parameter("convb", [128, d.FT], F32, isOutput=False)
    bdt = nc.declare_dram_parameter("bdt", [128, d.DT], F32, isOutput=False)
    dvec = nc.declare_dram_parameter("dvec", [128, d.DT], F32, isOutput=False)
    a_nd = nc.declare_dram_parameter("a_nd", [128, d.DT * d.NT], F32, isOutput=False)
    outT = nc.declare_dram_parameter("outT", [d.DM, d.L], F32, isOutput=True)

    NBC = 2 * d.NT  # B and C rows in x_dbl

    with tile.TileContext(nc) as tc:
        with (
            tc.tile_pool(name="singles", bufs=1) as sing,
            tc.tile_pool(name="big", bufs=2) as big,
            tc.tile_pool(name="ubuf", bufs=1) as ubuf,
            tc.tile_pool(name="work", bufs=2) as work,
            tc.tile_pool(name="scan", bufs=2) as scan,
            tc.tile_pool(name="bc", bufs=2) as bcp,
            tc.tile_pool(name="wstream", bufs=4) as wsp,
            tc.tile_pool(name="psA", bufs=2, space="PSUM") as psA,
            tc.tile_pool(name="psX", bufs=1, space="PSUM") as psX,
            tc.tile_pool(name="psD", bufs=2, space="PSUM") as psD,
            tc.tile_pool(name="psO", bufs=2, space="PSUM") as psO,
            tc.tile_pool(name="dram", bufs=2, space="DRAM") as drp,
        ):
            # ---- resident constants ----
            wx_sb = sing.tile([128, d.FT * 80], F32R)
            nc.sync.dma_start(out=wx_sb[:], in_=wx[:])
            wdt_sb = sing.tile([d.DR, d.DH], F32R)
            nc.sync.dma_start(out=wdt_sb[:], in_=wdt[:])
            convw_sb = sing.tile([128, d.FT * d.DC], F32)
            nc.sync.dma_start(out=convw_sb[:], in_=convw[:])
            convb_sb = sing.tile([128, d.FT], F32)
            nc.sync.dma_start(out=convb_sb[:], in_=convb[:])
            bdt_sb = sing.tile([128, d.DT], F32)
            nc.sync.dma_start(out=bdt_sb[:], in_=bdt[:])
            dvec_sb = sing.tile([128, d.DT], F32)
            nc.sync.dma_start(out=dvec_sb[:], in_=dvec[:])
            a_sb = sing.tile([128, d.DT * d.NT], F32)
            nc.sync.dma_start(out=a_sb[:], in_=a_nd[:])
            hstate = sing.tile([128, d.NT * d.DT], F32)
            nc.vector.memset(hstate[:], 0.0)
            xtail = sing.tile([128, d.FT * (d.DC - 1)], F32)

            CS = d.DC - 1  # conv history columns

            for p in range(d.NP):
                c0, c1 = p * LP, (p + 1) * LP

                xTp = big.tile([128, d.KT, LP], F32R)
                for k in range(d.KT):
                    nc.sync.dma_start(
                        out=xTp[:, k, :], in_=xT[k * 128:(k + 1) * 128, c0:c1]
                    )

                u_sb = ubuf.tile([128, d.DT, LP], F32R)
                delta_sb = big.tile([128, d.DT, LP], F32)
                du_sb = big.tile([128, d.DT, LP], F32)
                res_sb = big.tile([128, d.DT, LP], F32)
                y_sb = big.tile([128, d.DT, LP], F32R)

                xdbl_ps = psX.tile([80, LP], F32)

                # ---- stage A: in_proj (+conv+silu+x_dbl / +silu res) ----
                for m in range(d.MT):
                    ps = psA.tile([128, LP], F32)
                    for k in range(d.KT):
                        wblk = wsp.tile([128, 128], F32R)
                        nc.sync.dma_start(
                            out=wblk[:],
                            in_=w1[k * 128:(k + 1) * 128, m * 128:(m + 1) * 128],
                        )
                        nc.tensor.matmul(
                            ps[:], wblk[:],
                            xTp[:, k, :],
                            start=(k == 0), stop=(k == d.KT - 1),
                        )
                    if m < d.FT:
                        # depthwise causal conv over this d-tile
                        xp_t = work.tile([128, LP + CS], F32)
                        if p == 0:
                            nc.vector.memset(xp_t[:, 0:CS], 0.0)
                        else:
                            nc.vector.tensor_copy(
                                xp_t[:, 0:CS], xtail[:, m * CS:(m + 1) * CS]
                            )
                        nc.scalar.copy(xp_t[:, CS:CS + LP], ps[:])
                        if p < d.NP - 1:
                            nc.vector.tensor_copy(
                                xtail[:, m * CS:(m + 1) * CS], xp_t[:, LP:LP + CS]
                            )
                        c_t = work.tile([128, LP], F32)
                        nc.vector.tensor_scalar(
                            c_t[:], xp_t[:, 0:LP],
                            convw_sb[:, d.DC * m:d.DC * m + 1],
                            convb_sb[:, m:m + 1], OP.mult, OP.add,
                        )
                        for kk in range(1, d.DC):
                            nc.vector.scalar_tensor_tensor(
                                c_t[:], xp_t[:, kk:kk + LP],
                                convw_sb[:, d.DC * m + kk:d.DC * m + kk + 1],
                                c_t[:], OP.mult, OP.add,
                            )
                        if m < d.DT:
                            hdst = u_sb[:, m, :]
                        else:
                            h_t = work.tile([128, LP], F32R, tag="h_t")
                            hdst = h_t[:]
                        sg = work.tile([128, LP], F32, tag="sg")
                        nc.scalar.activation(sg[:], c_t[:], AF.Sigmoid)
                        nc.vector.tensor_mul(hdst, c_t[:], sg[:])
                        nc.tensor.matmul(
                            xdbl_ps[:],
                            wx_sb[:, m * 80:(m + 1) * 80],
                            hdst,
                            start=(m == 0), stop=(m == d.FT - 1),
                        )
                    else:
                        sgr = work.tile([128, LP], F32, tag="sg")
                        nc.scalar.activation(sgr[:], ps[:], AF.Sigmoid)
                        nc.vector.tensor_mul(
                            res_sb[:, m - d.FT, :], sgr[:], ps[:]
                        )

                # ---- x_dbl epilogue, B/C broadcast scratch ----
                xdbl_sb = big.tile([80, LP], F32R)
                nc.vector.tensor_copy(xdbl_sb[:], xdbl_ps[:])
                bc_dram = drp.tile([NBC, LP], F32)
                nc.sync.dma_start(
                    out=bc_dram[:], in_=xdbl_sb[d.DR:d.DR + NBC, :].bitcast(F32)
                )

                # ---- dt_proj + softplus + delta*u, y init = D*u ----
                for dt in range(d.DT):
                    psd = psD.tile([128, LP], F32)
                    nc.tensor.matmul(
                        psd[:],
                        wdt_sb[:, dt * 128:(dt + 1) * 128],
                        xdbl_sb[0:d.DR, :],
                        start=True, stop=True,
                    )
                    # softplus(z) = ln(1 + exp(z)); z = psd + b_dt stays tiny
                    sp_e = work.tile([128, LP], F32, tag="sg")
                    nc.scalar.activation(
                        sp_e[:], psd[:], AF.Exp, bias=bdt_sb[:, dt:dt + 1]
                    )
                    nc.scalar.activation(
                        delta_sb[:, dt, :], sp_e[:], AF.Ln, bias=1.0
                    )
                    nc.vector.tensor_mul(
                        du_sb[:, dt, :], delta_sb[:, dt, :], u_sb[:, dt, :]
                    )
                    nc.vector.tensor_scalar_mul(
                        y_sb[:, dt, :], u_sb[:, dt, :], dvec_sb[:, dt:dt + 1]
                    )

                # ---- selective scan ----
                for n in range(d.NT):
                    B_bc = bcp.tile([128, LP], F32)
                    srcB = bc_dram[n:n + 1, :]
                    nc.sync.dma_start(
                        out=B_bc[:],
                        in_=bass.AP(tensor=srcB.tensor, offset=srcB.offset,
                                    ap=[[0, 128]] + list(srcB.ap[1:])),
                    )
                    C_bc = bcp.tile([128, LP], F32)
                    srcC = bc_dram[d.NT + n:d.NT + n + 1, :]
                    nc.sync.dma_start(
                        out=C_bc[:],
                        in_=bass.AP(tensor=srcC.tensor, offset=srcC.offset,
                                    ap=[[0, 128]] + list(srcC.ap[1:])),
                    )
                    for dt in range(d.DT):
                        idx = n * d.DT + dt
                        dA = scan.tile([128, LP], F32)
                        nc.scalar.activation(
                            dA[:], delta_sb[:, dt, :], AF.Exp,
                            scale=a_sb[:, dt * d.NT + n:dt * d.NT + n + 1],
                        )
                        dBu = scan.tile([128, LP], F32)
                        nc.vector.tensor_mul(dBu[:], du_sb[:, dt, :], B_bc[:])
                        h_sc = scan.tile([128, LP], F32)
                        nc.vector.tensor_tensor_scan(
                            h_sc[:], dA[:], dBu[:],
                            hstate[:, idx:idx + 1], OP.mult, OP.add,
                        )
                        if p < d.NP - 1:
                            nc.gpsimd.tensor_copy(
                                hstate[:, idx:idx + 1], h_sc[:, LP - 1:LP]
                            )
                        t_y = scan.tile([128, LP], F32)
                        nc.vector.tensor_mul(t_y[:], h_sc[:], C_bc[:])
                        nc.vector.tensor_add(
                            y_sb[:, dt, :], y_sb[:, dt, :], t_y[:]
                        )

                # ---- gate ----
                for dt in range(d.DT):
                    nc.vector.tensor_mul(
                        y_sb[:, dt, :], y_sb[:, dt, :], res_sb[:, dt, :]
                    )

                # ---- out_proj (partial over local channels) ----
                for m in range(d.KT):
                    pso = psO.tile([128, LP], F32)
                    for k in range(d.DT):
                        woblk = wsp.tile([128, 128], F32R, tag="woblk")
                        nc.sync.dma_start(
                            out=woblk[:],
                            in_=wout[k * 128:(k + 1) * 128, m * 128:(m + 1) * 128],
                        )
                        nc.tensor.matmul(
                            pso[:], woblk[:],
                            y_sb[:, k, :],
                            start=(k == 0), stop=(k == d.DT - 1),
                        )
                    o_st = work.tile([128, LP], F32, tag="h_t")
                    nc.scalar.copy(o_st[:], pso[:])
                    nc.sync.dma_start(
                        out=outT[m * 128:(m + 1) * 128, c0:c1], in_=o_st[:]
                    )

    nc.finalize()
    return nc


def make_core_inputs(d: Dims, inputs: dict, b: int, j: int) -> dict:
    """Host-side slicing/packing of the full inputs for core (b, j)."""
    x = np.asarray(inputs["x"], np.float32)
    W_in = np.asarray(inputs["W_in"], np.float32)
    conv_w = np.asarray(inputs["conv_w"], np.float32)
    conv_b = np.asarray(inputs["conv_b"], np.float32)
    W_x = np.asarray(inputs["W_x"], np.float32)
    W_dt = np.asarray(inputs["W_dt"], np.float32)
    b_dt = np.asarray(inputs["b_dt"], np.float32)
    A_log = np.asarray(inputs["A_log"], np.float32)
    D = np.asarray(inputs["D"], np.float32)
    W_out = np.asarray(inputs["W_out"], np.float32)

    DH, DI = d.DH, d.DI
    loc = np.arange(j * DH, (j + 1) * DH)
    oth = np.arange((1 - j) * DH, (2 - j) * DH)
    perm = np.concatenate([loc, oth])

    A = -np.exp(A_log)  # (DI, DS)

    xT = np.ascontiguousarray(x[b].T)                       # (DM, L)
    w1 = np.ascontiguousarray(
        np.concatenate([W_in[perm].T, W_in[DI + loc].T], axis=1)
    )                                                        # (DM, DI+DH)
    wx = np.ascontiguousarray(
        W_x.T[perm].reshape(d.FT, 128, d.DS * 2 + d.DR)
        .transpose(1, 0, 2).reshape(128, d.FT * 80)
    )
    wdt = np.ascontiguousarray(W_dt[loc].T)                  # (DR, DH)
    wout = np.ascontiguousarray(W_out.T[loc])     # (DH, DM)
    convw = np.ascontiguousarray(
        conv_w[perm, 0, :].reshape(d.FT, 128, d.DC)
        .transpose(1, 0, 2).reshape(128, d.FT * d.DC)
    )
    convb = np.ascontiguousarray(
        conv_b[perm].reshape(d.FT, 128).T
    )
    bdt = np.ascontiguousarray(b_dt[loc].reshape(d.DT, 128).T)
    dvec = np.ascontiguousarray(D[loc].reshape(d.DT, 128).T)
    a_nd = np.ascontiguousarray(
        A[loc].reshape(d.DT, 128, d.NT).transpose(1, 0, 2)
        .reshape(128, d.DT * d.NT)
    )
    return dict(xT=xT, w1=w1, wx=wx, wdt=wdt, wout=wout, convw=convw,
                convb=convb, bdt=bdt, dvec=dvec, a_nd=a_nd)


_CACHE = {}


def _get_nc(d: Dims):
    key = (d.B, d.L, d.DM, d.DI, d.LP)
    if key not in _CACHE:
        _CACHE[key] = build_nc(d)
    return _CACHE[key]


def kernel(trace=False, **inputs) -> np.ndarray:
    d = Dims()
    nc = _get_nc(d)
    in_maps = [make_core_inputs(d, inputs, c // 2, c % 2) for c in range(8)]
    res = run_bass_kernel_spmd(nc, in_maps, list(range(8)), trace=trace)
    out = np.empty((d.B, d.L, d.DM), np.float32)
    for b in range(d.B):
        oT = res.results[2 * b]["outT"] + res.results[2 * b + 1]["outT"]
        out[b] = oT.T
    if trace:
        kernel.last_result = res
    return out


# revision 29
# speedup vs baseline: 1.4210x; 1.4210x over previous
"""Trainium2 Bass kernel for a Mamba-style selective-scan (SSM) layer.

Full shapes: x (4, 2048, 768); d_inner 1536, d_state 16, d_conv 4, dt_rank 48.

Sharding over 8 NeuronCores: core c handles batch b = c//2 and d_inner half
j = c%2 (768 channels).  The x-branch (in_proj x-part, depthwise conv,
x_proj) is duplicated across the two cores of a batch (it is needed in full
for B/C/delta); channels are permuted per core so that the local 768 channels
always occupy tiles 0..5 (keeps the program SPMD-uniform).  Each core's
out_proj partial (768, 2048) is summed host-side across the core pair.

On-chip layout is transposed: d on partitions, L on the free dim.  L is
processed in NP passes of LP columns; the selective scan runs as hardware
tensor_tensor_scan instructions (state = dA*state + dBu along the free dim),
one per (state index n, d-tile), with the [128, 96] running state chained
across passes via its `initial` operand.  Scan-space tensors are bf16 for
DVE 2x mode; matmul operands are float32r (full-rate fp32 path).
"""

import numpy as np

import concourse.bass as bass
from concourse import bacc
import concourse.mybir as mybir
import concourse.tile as tile
from concourse.bass_utils import run_bass_kernel_spmd

OP = mybir.AluOpType
AF = mybir.ActivationFunctionType
F32 = mybir.dt.float32
F32R = mybir.dt.float32r
BF16 = mybir.dt.bfloat16


class Dims:
    def __init__(self, B=4, L=2048, DM=768, DI=1536, DS=16, DC=4, DR=48, LP=1024):
        self.B, self.L, self.DM, self.DI = B, L, DM, DI
        self.DS, self.DC, self.DR, self.LP = DS, DC, DR, LP
        self.DH = DI // 2                  # local d_inner half per core
        self.NP = L // LP                  # number of L passes
        self.KT = DM // 128                # k-tiles of d_model
        self.FT = DI // 128                # full d_inner tiles
        self.DT = self.DH // 128           # local d_inner tiles
        self.MT = (DI + self.DH) // 128    # in_proj output tiles (x part + res)
        self.NT = DS                       # states
        self.CH = min(512, LP)             # PSUM chunk width
        self.NC = LP // self.CH            # PSUM chunks per pass
        assert DM % 128 == 0 and DI % 128 == 0 and L % LP == 0 and LP % self.CH == 0


def build_nc(d: Dims):
    nc = bacc.Bacc()
    LP, CH = d.LP, d.CH

    # ---- DRAM parameters (per-core inputs, host-prepacked) ----
    xTd = nc.declare_dram_parameter("xT", [d.NP, d.KT, 128, LP], F32R, isOutput=False)
    w1 = nc.declare_dram_parameter("w1", [d.MT, 128, d.KT * 128], F32R, isOutput=False)
    wx = nc.declare_dram_parameter("wx", [128, d.FT * 96], F32R, isOutput=False)
    wdt = nc.declare_dram_parameter("wdt", [d.DR, d.DH], F32R, isOutput=False)
    wout = nc.declare_dram_parameter("wout", [d.KT, 128, d.DT * 128], F32R,
                                     isOutput=False)
    convw = nc.declare_dram_parameter("convw", [128, d.FT * d.DC], F32, isOutput=False)
    convb = nc.declare_dram_parameter("convb", [128, d.FT], F32, isOutput=False)
    bdt = nc.declare_dram_parameter("bdt", [128, d.DT], F32, isOutput=False)
    dvec = nc.declare_dram_parameter("dvec", [128, d.DT], F32, isOutput=False)
    a_nd = nc.declare_dram_parameter("a_nd", [128, d.DT * d.NT], F32, isOutput=False)
    outT = nc.declare_dram_parameter("outT", [d.DM, d.L], F32, isOutput=True)

    NBC = 2 * d.NT  # B and C rows in x_dbl
    CS = d.DC - 1   # conv history columns

    with tile.TileContext(nc) as tc:
        with (
            tc.tile_pool(name="singles", bufs=1) as sing,
            tc.tile_pool(name="one", bufs=1) as one,
            tc.tile_pool(name="work", bufs=2) as work,
            tc.tile_pool(name="scan", bufs=2) as scan,
            tc.tile_pool(name="bc", bufs=2) as bcp,
            tc.tile_pool(name="wstream", bufs=2) as wsp,
            tc.tile_pool(name="psA", bufs=2, space="PSUM") as psA,
            tc.tile_pool(name="psX", bufs=1, space="PSUM") as psX,
            tc.tile_pool(name="psD", bufs=2, space="PSUM") as psD,
            tc.tile_pool(name="psO", bufs=2, space="PSUM") as psO,
            tc.tile_pool(name="dram", bufs=2, space="DRAM") as drp,
        ):
            # ---- resident constants ----
            wx_sb = sing.tile([128, d.FT * 96], F32R)
            nc.sync.dma_start(out=wx_sb[:], in_=wx[:])
            wdt_sb = sing.tile([d.DR, d.DH], F32R)
            nc.sync.dma_start(out=wdt_sb[:], in_=wdt[:])
            convw_sb = sing.tile([128, d.FT * d.DC], F32)
            nc.sync.dma_start(out=convw_sb[:], in_=convw[:])
            convb_sb = sing.tile([128, d.FT], F32)
            nc.sync.dma_start(out=convb_sb[:], in_=convb[:])
            bdt_sb = sing.tile([128, d.DT], F32)
            nc.sync.dma_start(out=bdt_sb[:], in_=bdt[:])
            dvec_sb = sing.tile([128, d.DT], F32)
            nc.sync.dma_start(out=dvec_sb[:], in_=dvec[:])
            a_sb = sing.tile([128, d.DT * d.NT], F32)
            nc.sync.dma_start(out=a_sb[:], in_=a_nd[:])
            hstate = sing.tile([128, d.NT * d.DT], F32)
            nc.vector.memset(hstate[:], 0.0)
            xtail = sing.tile([128, d.FT * CS], F32)

            for p in range(d.NP):
                c0 = p * LP

                xTp = one.tile([128, d.KT, LP], F32R)
                for k in range(d.KT):
                    nc.sync.dma_start(out=xTp[:, k, :], in_=xTd[p, k])

                u_sb = one.tile([128, d.DT, LP], F32R)
                delta_sb = one.tile([128, d.DT, LP], F32)
                du_sb = one.tile([128, d.DT, LP], BF16)
                res_sb = one.tile([128, d.DT, LP], BF16)
                y_sb = one.tile([128, d.DT, LP], F32R)
                xdbl_sb = one.tile([d.DR, LP], F32R)
                bcst = one.tile([64 + NBC, LP], BF16)
                psx_tiles = [
                    psX.tile([96, CH], F32, tag=f"psx{cc}", name=f"psx{cc}")
                    for cc in range(d.NC)
                ]

                # ---- stage A: in_proj (+conv+silu+x_dbl / +silu res) ----
                for m in range(d.MT):
                    wblk = wsp.tile([128, d.KT * 128], F32R)
                    nc.sync.dma_start(out=wblk[:], in_=w1[m])
                    if m < d.FT:
                        xp_t = work.tile([128, LP + CS], F32)
                        if p == 0:
                            nc.vector.memset(xp_t[:, 0:CS], 0.0)
                        else:
                            nc.vector.tensor_copy(
                                xp_t[:, 0:CS], xtail[:, m * CS:(m + 1) * CS]
                            )
                    for cc in range(d.NC):
                        ps = psA.tile([128, CH], F32)
                        for k in range(d.KT):
                            nc.tensor.matmul(
                                ps[:], wblk[:, k * 128:(k + 1) * 128],
                                xTp[:, k, cc * CH:(cc + 1) * CH],
                                start=(k == 0), stop=(k == d.KT - 1),
                            )
                        if m < d.FT:
                            nc.scalar.copy(
                                xp_t[:, CS + cc * CH:CS + (cc + 1) * CH], ps[:]
                            )
                        else:
                            sgr = work.tile([128, CH], F32, tag="sg")
                            nc.scalar.activation(sgr[:], ps[:], AF.Sigmoid)
                            nc.vector.tensor_mul(
                                res_sb[:, m - d.FT, cc * CH:(cc + 1) * CH],
                                sgr[:], ps[:],
                            )
                    if m < d.FT:
                        if p < d.NP - 1:
                            nc.vector.tensor_copy(
                                xtail[:, m * CS:(m + 1) * CS], xp_t[:, LP:LP + CS]
                            )
                        c_t = work.tile([128, LP], F32)
                        nc.vector.tensor_scalar(
                            c_t[:], xp_t[:, 0:LP],
                            convw_sb[:, d.DC * m:d.DC * m + 1],
                            convb_sb[:, m:m + 1], OP.mult, OP.add,
                        )
                        for kk in range(1, d.DC):
                            nc.vector.scalar_tensor_tensor(
                                c_t[:], xp_t[:, kk:kk + LP],
                                convw_sb[:, d.DC * m + kk:d.DC * m + kk + 1],
                                c_t[:], OP.mult, OP.add,
                            )
                        if m < d.DT:
                            hdst = u_sb[:, m, :]
                        else:
                            h_t = work.tile([128, LP], F32R, tag="h_t")
                            hdst = h_t[:]
                        sg = work.tile([128, LP], F32, tag="sg")
                        nc.scalar.activation(sg[:], c_t[:], AF.Sigmoid)
                        nc.vector.tensor_mul(hdst, c_t[:], sg[:])
                        for cc in range(d.NC):
                            nc.tensor.matmul(
                                psx_tiles[cc][:], wx_sb[:, m * 96:(m + 1) * 96],
                                hdst[:, cc * CH:(cc + 1) * CH],
                                start=(m == 0), stop=(m == d.FT - 1),
                            )
                            if m == d.FT - 1:
                                nc.vector.tensor_copy(
                                    xdbl_sb[:, cc * CH:(cc + 1) * CH],
                                    psx_tiles[cc][0:d.DR, :],
                                )
                                nc.vector.tensor_copy(
                                    bcst[64:64 + NBC, cc * CH:(cc + 1) * CH],
                                    psx_tiles[cc][64:64 + NBC, :],
                                )

                # ---- B/C rows -> DRAM broadcast scratch ----
                bc_dram = drp.tile([NBC, LP], BF16)
                nc.sync.dma_start(
                    out=bc_dram[:], in_=bcst[64:64 + NBC, :]
                )

                # ---- dt_proj + softplus(via exp/ln) + delta*u, y init = D*u ----
                for dt in range(d.DT):
                    sp_e = work.tile([128, LP], F32, tag="sg")
                    for cc in range(d.NC):
                        psd = psD.tile([128, CH], F32)
                        nc.tensor.matmul(
                            psd[:], wdt_sb[:, dt * 128:(dt + 1) * 128],
                            xdbl_sb[0:d.DR, cc * CH:(cc + 1) * CH],
                            start=True, stop=True,
                        )
                        nc.scalar.activation(
                            sp_e[:, cc * CH:(cc + 1) * CH], psd[:], AF.Exp,
                            bias=bdt_sb[:, dt:dt + 1],
                        )
                    nc.scalar.activation(
                        delta_sb[:, dt, :], sp_e[:], AF.Ln, bias=1.0
                    )
                    nc.vector.tensor_scalar_mul(
                        y_sb[:, dt, :], u_sb[:, dt, :], dvec_sb[:, dt:dt + 1]
                    )
                nc.vector.tensor_mul(du_sb[:], delta_sb[:], u_sb[:])

                # ---- selective scan ----
                for n in range(d.NT):
                    B_bc = bcp.tile([128, LP], BF16)
                    srcB = bc_dram[n:n + 1, :]
                    nc.sync.dma_start(
                        out=B_bc[:],
                        in_=bass.AP(tensor=srcB.tensor, offset=srcB.offset,
                                    ap=[[0, 128]] + list(srcB.ap[1:])),
                    )
                    C_bc = bcp.tile([128, LP], BF16)
                    srcC = bc_dram[d.NT + n:d.NT + n + 1, :]
                    nc.sync.dma_start(
                        out=C_bc[:],
                        in_=bass.AP(tensor=srcC.tensor, offset=srcC.offset,
                                    ap=[[0, 128]] + list(srcC.ap[1:])),
                    )
                    for dt in range(d.DT):
                        idx = n * d.DT + dt
                        dA = scan.tile([128, LP], BF16, tag="dA")
                        nc.scalar.activation(
                            dA[:], delta_sb[:, dt, :], AF.Exp,
                            scale=a_sb[:, dt * d.NT + n:dt * d.NT + n + 1],
                        )
                        dBu = scan.tile([128, LP], BF16, tag="dBu")
                        nc.vector.tensor_mul(dBu[:], du_sb[:, dt, :], B_bc[:])
                        h_sc = scan.tile([128, LP], BF16, tag="h_sc")
                        nc.vector.tensor_tensor_scan(
                            h_sc[:], dA[:], dBu[:],
                            hstate[:, idx:idx + 1], OP.mult, OP.add,
                        )
                        if p < d.NP - 1:
                            nc.vector.tensor_copy(
                                hstate[:, idx:idx + 1], h_sc[:, LP - 1:LP]
                            )
                        t_y = scan.tile([128, LP], BF16, tag="dA")
                        nc.vector.tensor_mul(t_y[:], h_sc[:], C_bc[:])
                        nc.gpsimd.tensor_add(
                            y_sb[:, dt, :], y_sb[:, dt, :], t_y[:]
                        )

                # ---- gate (y *= silu(res)), single 3D op ----
                nc.vector.tensor_mul(y_sb[:], y_sb[:], res_sb[:])

                # ---- out_proj (partial over local channels) ----
                for m in range(d.KT):
                    woblk = wsp.tile([128, d.DT * 128], F32R, tag="woblk")
                    nc.sync.dma_start(out=woblk[:], in_=wout[m])
                    o_st = work.tile([128, LP], F32, tag="h_t")
                    for cc in range(d.NC):
                        pso = psO.tile([128, CH], F32)
                        for k in range(d.DT):
                            nc.tensor.matmul(
                                pso[:], woblk[:, k * 128:(k + 1) * 128],
                                y_sb[:, k, cc * CH:(cc + 1) * CH],
                                start=(k == 0), stop=(k == d.DT - 1),
                            )
                        nc.scalar.copy(o_st[:, cc * CH:(cc + 1) * CH], pso[:])
                    nc.sync.dma_start(
                        out=outT[m * 128:(m + 1) * 128, c0:c0 + LP], in_=o_st[:]
                    )

    nc.finalize()
    return nc


def make_core_inputs(d: Dims, inputs: dict, b: int, j: int) -> dict:
    """Host-side slicing/packing of the full inputs for core (b, j)."""
    x = np.asarray(inputs["x"], np.float32)
    W_in = np.asarray(inputs["W_in"], np.float32)
    conv_w = np.asarray(inputs["conv_w"], np.float32)
    conv_b = np.asarray(inputs["conv_b"], np.float32)
    W_x = np.asarray(inputs["W_x"], np.float32)
    W_dt = np.asarray(inputs["W_dt"], np.float32)
    b_dt = np.asarray(inputs["b_dt"], np.float32)
    A_log = np.asarray(inputs["A_log"], np.float32)
    D = np.asarray(inputs["D"], np.float32)
    W_out = np.asarray(inputs["W_out"], np.float32)

    DH, DI = d.DH, d.DI
    loc = np.arange(j * DH, (j + 1) * DH)
    oth = np.arange((1 - j) * DH, (2 - j) * DH)
    perm = np.concatenate([loc, oth])

    A = -np.exp(A_log)  # (DI, DS)

    # x[b].T packed (NP, KT, 128, LP)
    xT = np.ascontiguousarray(
        x[b].T.reshape(d.KT, 128, d.NP, d.LP).transpose(2, 0, 1, 3)
    )
    # in_proj lhsT blocks: w1[m, r, k*128+c] = W_row[perm_or_res][:, dm]
    w1full = np.concatenate([W_in[perm].T, W_in[DI + loc].T], axis=1)  # (DM, MT*128)
    w1 = np.ascontiguousarray(
        w1full.reshape(d.KT, 128, d.MT, 128).transpose(2, 1, 0, 3)
        .reshape(d.MT, 128, d.KT * 128)
    )
    wxT = W_x.T[perm]                                    # (DI, DR+2DS)
    wx96 = np.zeros((d.DI, 96), np.float32)
    wx96[:, 0:d.DR] = wxT[:, 0:d.DR]                     # dlt rows -> 0:48
    wx96[:, 64:64 + 2 * d.DS] = wxT[:, d.DR:]            # B,C rows -> 64:96
    wx = np.ascontiguousarray(
        wx96.reshape(d.FT, 128, 96).transpose(1, 0, 2).reshape(128, d.FT * 96)
    )
    wdt = np.ascontiguousarray(W_dt[loc].T)                  # (DR, DH)
    woutT = W_out.T[loc]                                     # (DH, DM)
    wout = np.ascontiguousarray(
        woutT.reshape(d.DT, 128, d.KT, 128).transpose(2, 1, 0, 3)
        .reshape(d.KT, 128, d.DT * 128)
    )
    convw = np.ascontiguousarray(
        conv_w[perm, 0, :].reshape(d.FT, 128, d.DC)
        .transpose(1, 0, 2).reshape(128, d.FT * d.DC)
    )
    convb = np.ascontiguousarray(conv_b[perm].reshape(d.FT, 128).T)
    bdt = np.ascontiguousarray(b_dt[loc].reshape(d.DT, 128).T)
    dvec = np.ascontiguousarray(D[loc].reshape(d.DT, 128).T)
    a_nd = np.ascontiguousarray(
        A[loc].reshape(d.DT, 128, d.NT).transpose(1, 0, 2)
        .reshape(128, d.DT * d.NT)
    )
    return dict(xT=xT, w1=w1, wx=wx, wdt=wdt, wout=wout, convw=convw,
                convb=convb, bdt=bdt, dvec=dvec, a_nd=a_nd)


_CACHE = {}


def _get_nc(d: Dims):
    key = (d.B, d.L, d.DM, d.DI, d.LP)
    if key not in _CACHE:
        _CACHE[key] = build_nc(d)
    return _CACHE[key]


def kernel(trace=False, **inputs) -> np.ndarray:
    d = Dims()
    nc = _get_nc(d)
    in_maps = [make_core_inputs(d, inputs, c // 2, c % 2) for c in range(8)]
    res = run_bass_kernel_spmd(nc, in_maps, list(range(8)), trace=trace)
    out = np.empty((d.B, d.L, d.DM), np.float32)
    for b in range(d.B):
        oT = res.results[2 * b]["outT"] + res.results[2 * b + 1]["outT"]
        out[b] = oT.T
    if trace:
        kernel.last_result = res
    return out


# revision 32
# speedup vs baseline: 1.8936x; 1.3325x over previous
"""Trainium2 Bass kernel for a Mamba-style selective-scan (SSM) layer.

Full shapes: x (4, 2048, 768); d_inner 1536, d_state 16, d_conv 4, dt_rank 48.

Sharding over 8 NeuronCores: core c handles batch b = c//2 and d_inner half
j = c%2 (768 channels).  The x-branch (in_proj x-part, depthwise conv,
x_proj) is duplicated across the two cores of a batch (it is needed in full
for B/C/delta); channels are permuted per core so that the local 768 channels
always occupy tiles 0..5 (keeps the program SPMD-uniform).  Each core's
out_proj partial (768, 2048) is summed host-side across the core pair.

On-chip layout is transposed: d on partitions, L on the free dim.  L is
processed in NP passes of LP columns; the selective scan runs as hardware
tensor_tensor_scan instructions (state = dA*state + dBu along the free dim),
one per (state index n, d-tile), with the [128, 96] running state chained
across passes via its `initial` operand.  Scan-space tensors are bf16 for
DVE 2x mode; matmul operands are float32r (full-rate fp32 path).
"""

import numpy as np

import concourse.bass as bass
from concourse import bacc
import concourse.mybir as mybir
import concourse.tile as tile
from concourse.bass_utils import run_bass_kernel_spmd

OP = mybir.AluOpType
AF = mybir.ActivationFunctionType
F32 = mybir.dt.float32
F32R = mybir.dt.float32r
BF16 = mybir.dt.bfloat16


class Dims:
    def __init__(self, B=4, L=2048, DM=768, DI=1536, DS=16, DC=4, DR=48, LP=1024):
        self.B, self.L, self.DM, self.DI = B, L, DM, DI
        self.DS, self.DC, self.DR, self.LP = DS, DC, DR, LP
        self.DH = DI // 2                  # local d_inner half per core
        self.NP = L // LP                  # number of L passes
        self.KT = DM // 128                # k-tiles of d_model
        self.FT = DI // 128                # full d_inner tiles
        self.DT = self.DH // 128           # local d_inner tiles
        self.MT = (DI + self.DH) // 128    # in_proj output tiles (x part + res)
        self.NT = DS                       # states
        self.CH = min(512, LP)             # PSUM chunk width
        self.NC = LP // self.CH            # PSUM chunks per pass
        assert DM % 128 == 0 and DI % 128 == 0 and L % LP == 0 and LP % self.CH == 0


def build_nc(d: Dims):
    nc = bacc.Bacc()
    LP, CH = d.LP, d.CH

    # ---- DRAM parameters (per-core inputs, host-prepacked) ----
    xTd = nc.declare_dram_parameter("xT", [d.NP, d.KT, 128, LP], F32R, isOutput=False)
    w1 = nc.declare_dram_parameter("w1", [d.MT, 128, d.KT * 128], F32R, isOutput=False)
    wx = nc.declare_dram_parameter("wx", [128, d.FT * 96], F32R, isOutput=False)
    wdt = nc.declare_dram_parameter("wdt", [d.DR, d.DH], F32R, isOutput=False)
    wout = nc.declare_dram_parameter("wout", [d.KT, 128, d.DT * 128], F32R,
                                     isOutput=False)
    convw = nc.declare_dram_parameter("convw", [128, d.FT * d.DC], F32, isOutput=False)
    convb = nc.declare_dram_parameter("convb", [128, d.FT], F32, isOutput=False)
    bdt = nc.declare_dram_parameter("bdt", [128, d.DT], F32, isOutput=False)
    dvec = nc.declare_dram_parameter("dvec", [128, d.DT], F32, isOutput=False)
    a_nd = nc.declare_dram_parameter("a_nd", [128, d.DT * d.NT], F32, isOutput=False)
    ident = nc.declare_dram_parameter("ident", [128, 128], BF16, isOutput=False)
    outT = nc.declare_dram_parameter("outT", [d.DM, d.L], F32, isOutput=True)

    NBC = 2 * d.NT  # B and C rows in x_dbl
    CS = d.DC - 1   # conv history columns

    with tile.TileContext(nc) as tc:
        with (
            tc.tile_pool(name="singles", bufs=1) as sing,
            tc.tile_pool(name="one", bufs=1) as one,
            tc.tile_pool(name="work", bufs=2) as work,
            tc.tile_pool(name="scan", bufs=2) as scan,
            tc.tile_pool(name="bc", bufs=2) as bcp,
            tc.tile_pool(name="wstream", bufs=2) as wsp,
            tc.tile_pool(name="psA", bufs=2, space="PSUM") as psA,
            tc.tile_pool(name="psS", bufs=2, space="PSUM") as psS,
            tc.tile_pool(name="psY", bufs=1, space="PSUM") as psY,
            tc.tile_pool(name="dram", bufs=2, space="DRAM") as drp,
        ):
            # ---- resident constants ----
            wx_sb = sing.tile([128, d.FT * 96], F32R)
            nc.sync.dma_start(out=wx_sb[:], in_=wx[:])
            wdt_sb = sing.tile([d.DR, d.DH], F32R)
            nc.sync.dma_start(out=wdt_sb[:], in_=wdt[:])
            convw_sb = sing.tile([128, d.FT * d.DC], F32)
            nc.sync.dma_start(out=convw_sb[:], in_=convw[:])
            convb_sb = sing.tile([128, d.FT], F32)
            nc.sync.dma_start(out=convb_sb[:], in_=convb[:])
            bdt_sb = sing.tile([128, d.DT], F32)
            nc.sync.dma_start(out=bdt_sb[:], in_=bdt[:])
            dvec_sb = sing.tile([128, d.DT], F32)
            nc.sync.dma_start(out=dvec_sb[:], in_=dvec[:])
            a_sb = sing.tile([128, d.DT * d.NT], F32)
            nc.sync.dma_start(out=a_sb[:], in_=a_nd[:])
            id_sb = sing.tile([128, 128], BF16)
            nc.sync.dma_start(out=id_sb[:], in_=ident[:])
            hstate = sing.tile([128, d.NT * d.DT], F32)
            nc.vector.memset(hstate[:], 0.0)
            xtail = sing.tile([128, d.FT * CS], F32)

            for p in range(d.NP):
                c0 = p * LP

                xTp = one.tile([128, d.KT, LP], F32R)
                for k in range(d.KT):
                    nc.sync.dma_start(out=xTp[:, k, :], in_=xTd[p, k])

                u_sb = one.tile([128, d.DT, LP], F32R)
                delta_sb = one.tile([128, d.DT, LP], F32)
                du_sb = one.tile([128, d.DT, LP], BF16)
                res_sb = one.tile([128, d.DT, LP], BF16)
                y_sb = one.tile([128, d.DT, LP], F32R)
                xdbl_sb = one.tile([d.DR, LP], F32R)
                bcst = one.tile([64 + NBC, LP], BF16)
                psx_tiles = [
                    psS.tile([128, CH], F32, tag="sh", name=f"psx{cc}")
                    for cc in range(d.NC)
                ]

                # ---- stage A: in_proj (+conv+silu+x_dbl / +silu res) ----
                for m in range(d.MT):
                    wblk = wsp.tile([128, d.KT * 128], F32R)
                    nc.sync.dma_start(out=wblk[:], in_=w1[m])
                    if m < d.FT:
                        xp_t = work.tile([128, LP + CS], F32)
                        if p == 0:
                            nc.vector.memset(xp_t[:, 0:CS], 0.0)
                        else:
                            nc.vector.tensor_copy(
                                xp_t[:, 0:CS], xtail[:, m * CS:(m + 1) * CS]
                            )
                    for cc in range(d.NC):
                        ps = psA.tile([128, CH], F32)
                        for k in range(d.KT):
                            nc.tensor.matmul(
                                ps[:], wblk[:, k * 128:(k + 1) * 128],
                                xTp[:, k, cc * CH:(cc + 1) * CH],
                                start=(k == 0), stop=(k == d.KT - 1),
                            )
                        if m < d.FT:
                            nc.scalar.copy(
                                xp_t[:, CS + cc * CH:CS + (cc + 1) * CH], ps[:]
                            )
                        else:
                            sgr = work.tile([128, CH], F32, tag="sg")
                            nc.scalar.activation(sgr[:], ps[:], AF.Sigmoid)
                            nc.vector.tensor_mul(
                                res_sb[:, m - d.FT, cc * CH:(cc + 1) * CH],
                                sgr[:], ps[:],
                            )
                    if m < d.FT:
                        if p < d.NP - 1:
                            nc.vector.tensor_copy(
                                xtail[:, m * CS:(m + 1) * CS], xp_t[:, LP:LP + CS]
                            )
                        c_t = work.tile([128, LP], F32)
                        nc.vector.tensor_scalar(
                            c_t[:], xp_t[:, 0:LP],
                            convw_sb[:, d.DC * m:d.DC * m + 1],
                            convb_sb[:, m:m + 1], OP.mult, OP.add,
                        )
                        for kk in range(1, d.DC):
                            nc.vector.scalar_tensor_tensor(
                                c_t[:], xp_t[:, kk:kk + LP],
                                convw_sb[:, d.DC * m + kk:d.DC * m + kk + 1],
                                c_t[:], OP.mult, OP.add,
                            )
                        if m < d.DT:
                            hdst = u_sb[:, m, :]
                        else:
                            h_t = work.tile([128, LP], F32R, tag="h_t")
                            hdst = h_t[:]
                        sg = work.tile([128, LP], F32, tag="sg")
                        nc.scalar.activation(sg[:], c_t[:], AF.Sigmoid)
                        nc.vector.tensor_mul(hdst, c_t[:], sg[:])
                        for cc in range(d.NC):
                            nc.tensor.matmul(
                                psx_tiles[cc][0:96, :],
                                wx_sb[:, m * 96:(m + 1) * 96],
                                hdst[:, cc * CH:(cc + 1) * CH],
                                start=(m == 0), stop=(m == d.FT - 1),
                            )
                            if m == d.FT - 1:
                                nc.vector.tensor_copy(
                                    xdbl_sb[:, cc * CH:(cc + 1) * CH],
                                    psx_tiles[cc][0:d.DR, :],
                                )
                                nc.vector.tensor_copy(
                                    bcst[64:64 + NBC, cc * CH:(cc + 1) * CH],
                                    psx_tiles[cc][64:64 + NBC, :],
                                )

                # ---- B/C rows -> DRAM broadcast scratch ----
                bc_dram = drp.tile([NBC, LP], BF16)
                nc.sync.dma_start(
                    out=bc_dram[:], in_=bcst[64:64 + NBC, :]
                )

                # ---- dt_proj + softplus(via exp/ln) + delta*u, y init = D*u ----
                for dt in range(d.DT):
                    sp_e = work.tile([128, LP], F32, tag="sg")
                    for cc in range(d.NC):
                        psd = psS.tile([128, CH], F32, tag="sh", name="psd")
                        nc.tensor.matmul(
                            psd[:], wdt_sb[:, dt * 128:(dt + 1) * 128],
                            xdbl_sb[0:d.DR, cc * CH:(cc + 1) * CH],
                            start=True, stop=True,
                        )
                        nc.scalar.activation(
                            sp_e[:, cc * CH:(cc + 1) * CH], psd[:], AF.Exp,
                            bias=bdt_sb[:, dt:dt + 1],
                        )
                    nc.scalar.activation(
                        delta_sb[:, dt, :], sp_e[:], AF.Ln, bias=1.0
                    )
                nc.vector.tensor_mul(du_sb[:], delta_sb[:], u_sb[:])

                # ---- selective scan (dt-pairs; y accumulated on PE) ----
                for g in range(d.DT // 2):
                    yps = [
                        psY.tile([128, LP], F32, tag=f"y{i}", name=f"y{i}")
                        for i in range(2)
                    ]
                    for n in range(d.NT):
                        B_bc = bcp.tile([128, LP], BF16)
                        srcB = bc_dram[n:n + 1, :]
                        nc.sync.dma_start(
                            out=B_bc[:],
                            in_=bass.AP(tensor=srcB.tensor, offset=srcB.offset,
                                        ap=[[0, 128]] + list(srcB.ap[1:])),
                        )
                        C_bc = bcp.tile([128, LP], BF16)
                        srcC = bc_dram[d.NT + n:d.NT + n + 1, :]
                        nc.sync.dma_start(
                            out=C_bc[:],
                            in_=bass.AP(tensor=srcC.tensor, offset=srcC.offset,
                                        ap=[[0, 128]] + list(srcC.ap[1:])),
                        )
                        for i in range(2):
                            dt = 2 * g + i
                            idx = n * d.DT + dt
                            dA = scan.tile([128, LP], BF16, tag="dA")
                            nc.scalar.activation(
                                dA[:], delta_sb[:, dt, :], AF.Exp,
                                scale=a_sb[:, dt * d.NT + n:dt * d.NT + n + 1],
                            )
                            dBu = scan.tile([128, LP], BF16, tag="dBu")
                            nc.vector.tensor_mul(dBu[:], du_sb[:, dt, :], B_bc[:])
                            h_sc = scan.tile([128, LP], BF16, tag="h_sc")
                            nc.vector.tensor_tensor_scan(
                                h_sc[:], dA[:], dBu[:],
                                hstate[:, idx:idx + 1], OP.mult, OP.add,
                            )
                            if p < d.NP - 1:
                                nc.vector.tensor_copy(
                                    hstate[:, idx:idx + 1], h_sc[:, LP - 1:LP]
                                )
                            t_y = scan.tile([128, LP], BF16, tag="dA")
                            nc.vector.tensor_mul(t_y[:], h_sc[:], C_bc[:])
                            for cc in range(d.NC):
                                nc.tensor.matmul(
                                    yps[i][:, cc * CH:(cc + 1) * CH], id_sb[:],
                                    t_y[:, cc * CH:(cc + 1) * CH],
                                    start=(n == 0), stop=(n == d.NT - 1),
                                )
                    for i in range(2):
                        dt = 2 * g + i
                        nc.vector.scalar_tensor_tensor(
                            y_sb[:, dt, :], u_sb[:, dt, :],
                            dvec_sb[:, dt:dt + 1], yps[i][:],
                            OP.mult, OP.add,
                        )

                # ---- gate (y *= silu(res)), single 3D op ----
                nc.vector.tensor_mul(y_sb[:], y_sb[:], res_sb[:])

                # ---- out_proj (partial over local channels) ----
                for m in range(d.KT):
                    woblk = wsp.tile([128, d.DT * 128], F32R, tag="woblk")
                    nc.sync.dma_start(out=woblk[:], in_=wout[m])
                    o_st = work.tile([128, LP], F32, tag="h_t")
                    for cc in range(d.NC):
                        pso = psS.tile([128, CH], F32, tag="sh", name="pso")
                        for k in range(d.DT):
                            nc.tensor.matmul(
                                pso[:], woblk[:, k * 128:(k + 1) * 128],
                                y_sb[:, k, cc * CH:(cc + 1) * CH],
                                start=(k == 0), stop=(k == d.DT - 1),
                            )
                        nc.scalar.copy(o_st[:, cc * CH:(cc + 1) * CH], pso[:])
                    nc.sync.dma_start(
                        out=outT[m * 128:(m + 1) * 128, c0:c0 + LP], in_=o_st[:]
                    )

    nc.finalize()
    return nc


def make_core_inputs(d: Dims, inputs: dict, b: int, j: int) -> dict:
    """Host-side slicing/packing of the full inputs for core (b, j)."""
    x = np.asarray(inputs["x"], np.float32)
    W_in = np.asarray(inputs["W_in"], np.float32)
    conv_w = np.asarray(inputs["conv_w"], np.float32)
    conv_b = np.asarray(inputs["conv_b"], np.float32)
    W_x = np.asarray(inputs["W_x"], np.float32)
    W_dt = np.asarray(inputs["W_dt"], np.float32)
    b_dt = np.asarray(inputs["b_dt"], np.float32)
    A_log = np.asarray(inputs["A_log"], np.float32)
    D = np.asarray(inputs["D"], np.float32)
    W_out = np.asarray(inputs["W_out"], np.float32)

    DH, DI = d.DH, d.DI
    loc = np.arange(j * DH, (j + 1) * DH)
    oth = np.arange((1 - j) * DH, (2 - j) * DH)
    perm = np.concatenate([loc, oth])

    A = -np.exp(A_log)  # (DI, DS)

    # x[b].T packed (NP, KT, 128, LP)
    xT = np.ascontiguousarray(
        x[b].T.reshape(d.KT, 128, d.NP, d.LP).transpose(2, 0, 1, 3)
    )
    # in_proj lhsT blocks: w1[m, r, k*128+c] = W_row[perm_or_res][:, dm]
    w1full = np.concatenate([W_in[perm].T, W_in[DI + loc].T], axis=1)  # (DM, MT*128)
    w1 = np.ascontiguousarray(
        w1full.reshape(d.KT, 128, d.MT, 128).transpose(2, 1, 0, 3)
        .reshape(d.MT, 128, d.KT * 128)
    )
    wxT = W_x.T[perm]                                    # (DI, DR+2DS)
    wx96 = np.zeros((d.DI, 96), np.float32)
    wx96[:, 0:d.DR] = wxT[:, 0:d.DR]                     # dlt rows -> 0:48
    wx96[:, 64:64 + 2 * d.DS] = wxT[:, d.DR:]            # B,C rows -> 64:96
    wx = np.ascontiguousarray(
        wx96.reshape(d.FT, 128, 96).transpose(1, 0, 2).reshape(128, d.FT * 96)
    )
    wdt = np.ascontiguousarray(W_dt[loc].T)                  # (DR, DH)
    woutT = W_out.T[loc]                                     # (DH, DM)
    wout = np.ascontiguousarray(
        woutT.reshape(d.DT, 128, d.KT, 128).transpose(2, 1, 0, 3)
        .reshape(d.KT, 128, d.DT * 128)
    )
    convw = np.ascontiguousarray(
        conv_w[perm, 0, :].reshape(d.FT, 128, d.DC)
        .transpose(1, 0, 2).reshape(128, d.FT * d.DC)
    )
    convb = np.ascontiguousarray(conv_b[perm].reshape(d.FT, 128).T)
    bdt = np.ascontiguousarray(b_dt[loc].reshape(d.DT, 128).T)
    dvec = np.ascontiguousarray(D[loc].reshape(d.DT, 128).T)
    a_nd = np.ascontiguousarray(
        A[loc].reshape(d.DT, 128, d.NT).transpose(1, 0, 2)
        .reshape(128, d.DT * d.NT)
    )
    import ml_dtypes
    ident = np.eye(128, dtype=ml_dtypes.bfloat16)
    return dict(xT=xT, w1=w1, wx=wx, wdt=wdt, wout=wout, convw=convw,
                convb=convb, bdt=bdt, dvec=dvec, a_nd=a_nd, ident=ident)


_CACHE = {}


def _get_nc(d: Dims):
    key = (d.B, d.L, d.DM, d.DI, d.LP)
    if key not in _CACHE:
        _CACHE[key] = build_nc(d)
    return _CACHE[key]


def kernel(trace=False, **inputs) -> np.ndarray:
    d = Dims()
    nc = _get_nc(d)
    in_maps = [make_core_inputs(d, inputs, c // 2, c % 2) for c in range(8)]
    res = run_bass_kernel_spmd(nc, in_maps, list(range(8)), trace=trace)
    out = np.empty((d.B, d.L, d.DM), np.float32)
    for b in range(d.B):
        oT = res.results[2 * b]["outT"] + res.results[2 * b + 1]["outT"]
        out[b] = oT.T
    if trace:
        kernel.last_result = res
    return out
